# revision 1
# baseline (speedup 1.0000x reference)
"""Multi-head causal attention (B=2, T=2048, D=2048, H=16) on 8 trn2 NeuronCores.

Sharding: tensor-parallel over heads (2 heads/core). x^T is replicated, W_qkv
column-sliced and W_out row-sliced per core; each core computes a full-shape
partial of the output projection and the host sums the 8 partials (+ b_out).

All big matmuls run as float32r (fp32 storage, reduced-precision PE multiply
at full rate). Causal structure: only lower-triangular score tiles are
computed. Softmax runs without max-subtraction (scores provably < ~25, safe
in fp32) and without normalizing P: scores are computed transposed [k, q],
exponentiated, causally zeroed on diagonal blocks (GPSIMD affine_select),
and fed straight to P@V; row-sums accumulate alongside via a ones-column
matmul and the normalization happens once at the O^T eviction.
"""

import math
import os

import numpy as np

import concourse.bass as bass
import concourse.mybir as mybir
import concourse.tile as tile
from concourse import bacc
from concourse.bass_utils import run_bass_kernel_spmd
from concourse.masks import make_identity

B, T, D_IN, D_MODEL, H = 2, 2048, 2048, 2048, 16
DH = 128
NCORES = 8
HPC = H // NCORES  # heads per core
BT = B * T
SCALE = 1.0 / math.sqrt(DH)

F32 = mybir.dt.float32
F32R = mybir.dt.float32r
AF = mybir.ActivationFunctionType
ALU = mybir.AluOpType

TOKT = 512             # stage-1 token tile
NTT = T // TOKT        # token tiles per batch (4)
NDCH = D_IN // 128     # d_in contraction chunks (16)
NQ = T // 128          # 128-token chunks per batch (16)
NJ = T // 512          # q 512-tiles per batch (4)
NFT = D_MODEL // 512   # output feature tiles (4)


def build_nc(debug=False, reps=1):
    nc = bacc.Bacc("TRN2", target_bir_lowering=False, debug=False,
                   num_devices=NCORES)

    xT = nc.dram_tensor("xT", [D_IN, BT], F32R, kind="ExternalInput")
    wq = nc.dram_tensor("wq", [D_IN, HPC * DH], F32R, kind="ExternalInput")
    wk = nc.dram_tensor("wk", [D_IN, HPC * DH], F32R, kind="ExternalInput")
    wv = nc.dram_tensor("wv", [D_IN, HPC * DH], F32R, kind="ExternalInput")
    bq = nc.dram_tensor("bq", [HPC * DH], F32, kind="ExternalInput")
    bk = nc.dram_tensor("bk", [HPC * DH], F32, kind="ExternalInput")
    bv = nc.dram_tensor("bv", [HPC * DH], F32, kind="ExternalInput")
    wo = nc.dram_tensor("wo", [HPC * DH, D_MODEL], F32R, kind="ExternalInput")
    cosT = nc.dram_tensor("cosT", [DH, T], F32, kind="ExternalInput")
    sinTs = nc.dram_tensor("sinTs", [DH, T], F32, kind="ExternalInput")
    y = nc.dram_tensor("y", [BT, D_MODEL], F32, kind="ExternalOutput")

    dbg = {}
    if debug:
        dbg["qT"] = nc.dram_tensor("dbg_qT", [HPC, B, DH, T], F32, kind="ExternalOutput")
        dbg["kT"] = nc.dram_tensor("dbg_kT", [HPC, B, DH, T], F32, kind="ExternalOutput")
        dbg["v"] = nc.dram_tensor("dbg_v", [B, T, HPC * DH], F32, kind="ExternalOutput")
        dbg["ot"] = nc.dram_tensor("dbg_ot", [B, HPC, DH, T], F32, kind="ExternalOutput")

    with tile.TileContext(nc) as tc:
        with (
            tc.tile_pool(name="persist", bufs=1) as pp,
            tc.tile_pool(name="weights", bufs=1) as wp,
            tc.tile_pool(name="qkv", bufs=1) as qp,
        ):
            # ---- per-core weights, quarter 0 first (unblocks first matmuls)
            wq_sb = wp.tile([128, NDCH, HPC * DH], F32R, name="wq_sb")
            wk_sb = wp.tile([128, NDCH, HPC * DH], F32R, name="wk_sb")
            wv_sb = wp.tile([128, NDCH, HPC * DH], F32R, name="wv_sb")

            def _w_quarter(hf):
                for t_, d_ in ((wq_sb, wq), (wk_sb, wk), (wv_sb, wv)):
                    nc.sync.dma_start(
                        t_[:, hf * (NDCH // 4):(hf + 1) * (NDCH // 4), :],
                        d_.ap()[hf * (D_IN // 4):(hf + 1) * (D_IN // 4), :]
                        .rearrange("(c p) f -> p c f", p=128))

            _w_quarter(0)

            # prefetch the very first x^T quarter so tau-0 matmuls start early
            # (skipped in the repeated timing build: a tile allocated outside
            # the loop but read inside would pin its pool slot across reps)
            xs_cm = tc.tile_pool(name="xs", bufs=3)
            xs = xs_cm.__enter__()
            xt00 = None
            if reps == 1:
                xt00 = xs.tile([128, 4, TOKT], F32R, name="xt")
                nc.sync.dma_start(
                    xt00[:],
                    xT.ap()[0:512, 0:TOKT].rearrange("(c p) t -> p c t", p=128))

            # ---- constants (needed ~15us in, after the first accumulations)
            cosT_sb = pp.tile([DH, T], F32, name="cosT_sb")
            sinTs_sb = pp.tile([DH, T], F32, name="sinTs_sb")
            nc.sync.dma_start(cosT_sb[:], cosT.ap())
            nc.sync.dma_start(sinTs_sb[:], sinTs.ap())
            ones1 = pp.tile([1, 128], F32, name="ones1")
            nc.gpsimd.memset(ones1[:], 1.0)
            onescol = pp.tile([128, 1], F32, name="onescol")
            nc.gpsimd.memset(onescol[:], 1.0)
            onescol_r = pp.tile([128, 1], F32R, name="onescol_r")
            nc.scalar.copy(onescol_r[:], onescol[:])
            ident = pp.tile([128, 128], F32, name="ident")
            make_identity(nc, ident[:])
            bqt = pp.tile([128, HPC], F32, name="bqt")
            bkt = pp.tile([128, HPC], F32, name="bkt")
            bvt = pp.tile([128, HPC], F32, name="bvt")
            nc.sync.dma_start(bqt[:], bq.ap().rearrange("(h d) -> d h", d=DH))
            nc.sync.dma_start(bkt[:], bk.ap().rearrange("(h d) -> d h", d=DH))
            nc.sync.dma_start(bvt[:], bv.ap().rearrange("(h d) -> d h", d=DH))

            for hf in range(1, 4):
                _w_quarter(hf)

            # ---- per-batch Q^T/K^T/V and O^T buffers -----------------------
            qT_sb = [qp.tile([DH, T], F32R, name=f"qT{h}") for h in range(HPC)]
            kT_sb = [qp.tile([DH, T], F32R, name=f"kT{h}") for h in range(HPC)]
            v_sb = qp.tile([128, NQ, HPC * DH], F32R, name="v_sb")
            ot_sb = [[pp.tile([DH, T], F32R, name=f"ot{b}_{h}") for h in range(HPC)]
                     for b in range(B)]

            import contextlib
            rep_ctx = (tc.For_i(0, reps, 1, hint_engines=(
                mybir.EngineType.PE, mybir.EngineType.Activation,
                mybir.EngineType.DVE, mybir.EngineType.Pool,
                mybir.EngineType.SP))
                if reps > 1 else contextlib.nullcontext())
            with rep_ctx:
                _emit_body(nc, tc, xT, wq_sb, wk_sb, wv_sb, bqt, bkt, bvt,
                           cosT_sb, sinTs_sb, qT_sb, kT_sb, v_sb, ot_sb,
                           wo, y, ones1, onescol_r, ident, dbg, xs, xt00)
            xs_cm.__exit__(None, None, None)
    nc.compile()
    return nc


def _emit_body(nc, tc, xT, wq_sb, wk_sb, wv_sb, bqt, bkt, bvt, cosT_sb,
               sinTs_sb, qT_sb, kT_sb, v_sb, ot_sb, wo, y, ones1,
               onescol_r, ident, dbg, xs, xt00):
    wop_cm = tc.tile_pool(name="wo_p", bufs=1)
    wop = wop_cm.__enter__()
    wo_sb = None
    ypools = {}

    def emit_y(b):
        if not ypools:
            ypools["yp_cm"] = tc.tile_pool(name="y_p", bufs=4)
            ypools["yp"] = ypools["yp_cm"].__enter__()
            ypools["yps_cm"] = tc.tile_pool(name="y_ps", bufs=4, space="PSUM")
            ypools["yps"] = ypools["yps_cm"].__enter__()
        yp, yps = ypools["yp"], ypools["yps"]
        for tt in range(NQ):
            for ft in range(NFT):
                ps = yps.tile([128, 512], F32, name="y_acc")
                for h in range(HPC):
                    nc.tensor.matmul(
                        ps[:], ot_sb[b][h][:, tt * 128:(tt + 1) * 128],
                        wo_sb[:, h, ft * 512:(ft + 1) * 512],
                        start=(h == 0), stop=(h == HPC - 1))
                yt = yp.tile([128, 512], F32, name="y_t")
                nc.scalar.copy(yt[:], ps[:])
                nc.sync.dma_start(
                    y.ap()[b * T + tt * 128:b * T + (tt + 1) * 128,
                           ft * 512:(ft + 1) * 512],
                    yt[:])

    for b in range(B):
        _stage1(nc, tc, b, xT, wq_sb, wk_sb, wv_sb, bqt, bkt, bvt,
                cosT_sb, sinTs_sb, qT_sb, kT_sb, v_sb, ident, xs,
                xt00 if b == 0 else None)
        if dbg:
            for h in range(HPC):
                nc.sync.dma_start(dbg["qT"].ap()[h, b], qT_sb[h][:].bitcast(F32))
                nc.sync.dma_start(dbg["kT"].ap()[h, b], kT_sb[h][:].bitcast(F32))
            nc.sync.dma_start(
                dbg["v"].ap()[b].rearrange("(c p) f -> p c f", p=128),
                v_sb[:].bitcast(F32))
        _stage2(nc, tc, b, qT_sb, kT_sb, v_sb, ones1, onescol_r,
                ot_sb, dbg)
        if b == 0:
            # prefetch W_out during the second batch's compute
            wo_sb = wop.tile([128, HPC, D_MODEL], F32R, name="wo_sb")
            nc.sync.dma_start(wo_sb[:],
                               wo.ap().rearrange("(h p) f -> p h f", p=128))

    if dbg:
        for bb in range(B):
            for h in range(HPC):
                nc.sync.dma_start(dbg["ot"].ap()[bb, h],
                                  ot_sb[bb][h][:].bitcast(F32))
    emit_y(0)
    emit_y(1)
    ypools["yps_cm"].__exit__(None, None, None)
    ypools["yp_cm"].__exit__(None, None, None)
    wop_cm.__exit__(None, None, None)


def _stage1(nc, tc, b, xT, wq_sb, wk_sb, wv_sb, bqt, bkt, bvt,
            cosT_sb, sinTs_sb, qT_sb, kT_sb, v_sb, ident, xs, xt00):
    """QKV projection + RoPE for batch b: fills qT_sb/kT_sb/v_sb.

    Loop nest is d_in-chunk-outer so each x^T quarter-tile is touched once.
    q/k/v are all computed transposed ([feat, tok], N=512, weight loads fully
    hidden); V is then rotated back to natural [tok, feat] layout with PE
    transposes so it can serve as the stationary operand of P@V.
    """
    with (
        tc.tile_pool(name="st", bufs=2) as st,
        tc.tile_pool(name="vt", bufs=2) as vtp,
        tc.tile_pool(name="ps_qk", bufs=4, space="PSUM") as psqk,
        tc.tile_pool(name="ps_v", bufs=2, space="PSUM") as psv,
        tc.tile_pool(name="ps_tr", bufs=2, space="PSUM") as pstr,
    ):
        for tau in range(NTT):
            pos = tau * TOKT
            gtok = b * T + pos
            accs = [psqk.tile([128, TOKT], F32, name="qk_acc") for _ in range(4)]
            accvT = [psv.tile([128, TOKT], F32, name="vT_acc") for _ in range(2)]
            for quarter in range(4):
                if tau == 0 and quarter == 0 and xt00 is not None:
                    xt = xt00
                else:
                    xt = xs.tile([128, 4, TOKT], F32R, name="xt")
                    nc.sync.dma_start(
                        xt[:],
                        xT.ap()[quarter * 512:(quarter + 1) * 512,
                                gtok:gtok + TOKT]
                        .rearrange("(c p) t -> p c t", p=128))
                for cl in range(4):
                    c = quarter * 4 + cl
                    for fi, (wsb, hh) in enumerate(
                            ((wq_sb, 0), (wq_sb, 1), (wk_sb, 0), (wk_sb, 1))):
                        nc.tensor.matmul(
                            accs[fi][:], wsb[:, c, hh * DH:(hh + 1) * DH],
                            xt[:, cl, :],
                            start=(c == 0), stop=(c == NDCH - 1))
                    for hh in range(HPC):
                        nc.tensor.matmul(
                            accvT[hh][:], wv_sb[:, c, hh * DH:(hh + 1) * DH],
                            xt[:, cl, :],
                            start=(c == 0), stop=(c == NDCH - 1))
            # q/k evictions with bias (split ACT/DVE), then RoPE on DVE
            for fi, (bias, dest, hh) in enumerate(
                    ((bqt, qT_sb, 0), (bqt, qT_sb, 1),
                     (bkt, kT_sb, 0), (bkt, kT_sb, 1))):
                stg = st.tile([128, TOKT], F32, name="stg")
                if fi < 2:
                    nc.scalar.activation(stg[:], accs[fi][:], AF.Identity,
                                         bias=bias[:, hh:hh + 1], scale=1.0)
                else:
                    nc.vector.tensor_scalar_add(stg[:], accs[fi][:],
                                                bias[:, hh:hh + 1])
                rot = st.tile([128, TOKT], F32, name="rot")
                nc.scalar.copy(rot[0:64, :], stg[64:128, :])
                nc.scalar.copy(rot[64:128, :], stg[0:64, :])
                nc.vector.tensor_tensor(
                    stg[:], stg[:], cosT_sb[:, pos:pos + TOKT], ALU.mult)
                nc.vector.tensor_tensor(
                    rot[:], rot[:], sinTs_sb[:, pos:pos + TOKT], ALU.mult)
                nc.vector.tensor_tensor(
                    dest[hh][:, pos:pos + TOKT], stg[:], rot[:], ALU.add)
            # V: evict V^T with bias, then PE-transpose back to natural layout
            # (transpose runs in plain fp32 — the f32r LDW path is broken in
            # walrus codegen; rounding to f32r happens in the PSUM eviction)
            for hh in range(HPC):
                vt = vtp.tile([128, TOKT], F32, name="vt")
                nc.scalar.activation(vt[:], accvT[hh][:], AF.Identity,
                                     bias=bvt[:, hh:hh + 1], scale=1.0)
                for ts in range(4):
                    tr = pstr.tile([128, 128], F32, name="tr")
                    nc.tensor.transpose(tr[:], vt[:, ts * 128:(ts + 1) * 128],
                                        ident[:])
                    nc.scalar.copy(
                        v_sb[:, (pos // 128) + ts, hh * DH:(hh + 1) * DH], tr[:])


def _stage2(nc, tc, b, qT_sb, kT_sb, v_sb, ones1, onescol_r, ot_sb, dbg):
    """Causal attention for batch b, both heads interleaved: fills ot_sb[b].

    Single pass per tile: S^T -> exp -> causal zero (diag blocks, GPSIMD) ->
    P@V accumulation + ones-matmul row-sum accumulation; O^T normalized by
    1/rowsum (PE-broadcast along partitions) during eviction. The two heads
    alternate per (j, kk) step so one head's exp latency hides under the
    other head's matmuls.
    """
    with (
        tc.tile_pool(name="spsB", bufs=3, space="PSUM") as spsB,
        tc.tile_pool(name="rps", bufs=2, space="PSUM") as rps,
        tc.tile_pool(name="ops", bufs=2, space="PSUM") as ops,
        tc.tile_pool(name="scr", bufs=2) as scr,
        tc.tile_pool(name="pt_p", bufs=5) as ptp,
    ):
        for j in range(NJ):
            nkk = 4 * j + 4
            rp = [rps.tile([1, 512], F32, name="r_ps") for _ in range(HPC)]
            op = [ops.tile([128, 512], F32, name="o_ps") for _ in range(HPC)]
            for kk in range(nkk):
                for h in range(HPC):
                    qT, kT = qT_sb[h], kT_sb[h]
                    sp = spsB.tile([128, 512], F32, name="st_ps")
                    nc.tensor.matmul(sp[:], kT[:, kk * 128:(kk + 1) * 128],
                                     qT[:, j * 512:(j + 1) * 512],
                                     start=True, stop=True)
                    pt = ptp.tile([128, 512], F32R, name="pt")
                    nc.scalar.activation(pt[:], sp[:], AF.Exp, bias=0.0,
                                         scale=SCALE)
                    if kk // 4 == j:
                        # zero entries with q < k: keep where f - p - off >= 0
                        nc.gpsimd.affine_select(
                            out=pt[:], in_=pt[:], compare_op=ALU.is_ge,
                            fill=0.0, base=-(kk % 4) * 128, pattern=[[1, 512]],
                            channel_multiplier=-1)
                    nc.tensor.matmul(op[h][:],
                                     v_sb[:, kk, h * DH:(h + 1) * DH],
                                     pt[:], start=(kk == 0),
                                     stop=(kk == nkk - 1))
                    nc.tensor.matmul(rp[h][:], onescol_r[:], pt[:],
                                     start=(kk == 0), stop=(kk == nkk - 1))
            # rowsum -> reciprocal -> broadcast across partitions -> evict
            for h in range(HPC):
                rrow_inv = scr.tile([1, 512], F32, name="rrow_inv")
                nc.vector.reciprocal(rrow_inv[:], rp[h][:])
                rb_ps = spsB.tile([128, 512], F32, name="st_ps", tag="st_ps")
                nc.tensor.matmul(rb_ps[:], ones1[:], rrow_inv[:],
                                 start=True, stop=True)
                rb = scr.tile([128, 512], F32, name="rb")
                nc.scalar.copy(rb[:], rb_ps[:])
                nc.vector.tensor_tensor(ot_sb[b][h][:, j * 512:(j + 1) * 512],
                                        op[h][:], rb[:], ALU.mult)


_CACHE = {}


def _get_nc():
    if "nc" not in _CACHE:
        _CACHE["nc"] = build_nc(debug=bool(int(os.environ.get("KERNEL_DEBUG", "0"))))
    return _CACHE["nc"]


def _host_prep(x, W_qkv, b_qkv, W_out, mask):
    xT = np.ascontiguousarray(x.reshape(BT, D_IN).T)
    Wr = W_qkv.reshape(D_IN, H, 3, DH)
    br = b_qkv.reshape(H, 3, DH)
    # RoPE tables, transposed, sign-folded (rows 0:64 of sinTs negated)
    inv_freq = (1.0 / (10000.0 ** (np.arange(0, DH, 2, dtype=np.float32) / DH))).astype(np.float32)
    tpos = np.arange(T, dtype=np.float32)
    freqs = tpos[:, None] * inv_freq[None, :]              # (T, 64)
    emb = np.concatenate([freqs, freqs], axis=-1)          # (T, 128)
    cosT = np.ascontiguousarray(np.cos(emb).astype(np.float32).T)
    sinT = np.sin(emb).astype(np.float32).T
    sinTs = sinT.copy()
    sinTs[0:64] = -sinTs[0:64]
    sinTs = np.ascontiguousarray(sinTs)

    in_maps = []
    for i in range(NCORES):
        hs = [HPC * i + k for k in range(HPC)]
        in_maps.append({
            "xT": xT,
            "wq": np.ascontiguousarray(Wr[:, hs, 0, :].reshape(D_IN, HPC * DH)),
            "wk": np.ascontiguousarray(Wr[:, hs, 1, :].reshape(D_IN, HPC * DH)),
            "wv": np.ascontiguousarray(Wr[:, hs, 2, :].reshape(D_IN, HPC * DH)),
            "bq": np.ascontiguousarray(br[hs, 0, :].reshape(HPC * DH)),
            "bk": np.ascontiguousarray(br[hs, 1, :].reshape(HPC * DH)),
            "bv": np.ascontiguousarray(br[hs, 2, :].reshape(HPC * DH)),
            "wo": np.ascontiguousarray(W_out[hs[0] * DH:(hs[-1] + 1) * DH, :]),
            "cosT": cosT,
            "sinTs": sinTs,
        })
    return in_maps


def kernel(x, W_qkv, b_qkv, W_out, b_out, mask):
    x = np.asarray(x, dtype=np.float32)
    in_maps = _host_prep(np.asarray(x), np.asarray(W_qkv), np.asarray(b_qkv),
                         np.asarray(W_out), np.asarray(mask))
    nc = _get_nc()
    res = run_bass_kernel_spmd(nc, in_maps, core_ids=list(range(NCORES)))
    out = res.results[0]["y"].copy()
    for i in range(1, NCORES):
        out += res.results[i]["y"]
    out += np.asarray(b_out, dtype=np.float32)[None, :]
    return out.reshape(B, T, D_MODEL).astype(np.float32)



# revision 8
# speedup vs baseline: 1.3436x; 1.3436x over previous
"""Multi-head causal attention (B=2, T=2048, D=2048, H=16) on 8 trn2 NeuronCores.

Sharding: tensor-parallel over heads (2 heads/core). x^T replicated, W_qkv
column-sliced and W_out row-sliced per core; each core computes a full-shape
bf16 partial of the output projection and the host sums the 8 partials
(+ b_out) in f32.

v2 design (vs f32r baseline):
- All matmul operands bf16 (full PE rate at any free size; f32 PSUM accum).
  Halves DMA + SBUF traffic; rel-err budget (2e-2) allows it.
- V computed in natural [tok, feat] layout directly (x-chunk stationary,
  W_v moving) -- no PE transposes.
- Softmax row-sums accumulated on DVE (bf16 Z += P tile, 2x perf mode)
  instead of per-tile ones-matmuls on PE; one ones-matmul per (j, head)
  reduces Z across partitions.
- Diagonal score tiles trimmed: only the valid q-range is matmul'd/exp'd,
  and the causal affine_select shrinks to one [128,128] triangular chunk.
- Output projection tiles are emitted interleaved into stage2 as each
  j-block's O^T finishes, keeping PE busy while ACT runs the exps; y is
  evicted to bf16 (rotating ACT/DVE/Pool) and DMA'd per tile.
"""

import math
import os

import numpy as np

import concourse.bass as bass
import concourse.mybir as mybir
import concourse.tile as tile
from concourse import bacc
from concourse.bass_utils import run_bass_kernel_spmd

B, T, D_IN, D_MODEL, H = 2, 2048, 2048, 2048, 16
DH = 128
NCORES = 8
HPC = H // NCORES  # heads per core
BT = B * T
SCALE = 1.0 / math.sqrt(DH)

F32 = mybir.dt.float32
F32R = mybir.dt.float32r
BF16 = mybir.dt.bfloat16
AF = mybir.ActivationFunctionType
ALU = mybir.AluOpType

TOKT = 512             # token tile (q-window, stage-1 tile)
NTT = T // TOKT        # token tiles per batch (4)
NDCH = D_IN // 128     # d_in contraction chunks (16)
NQ = T // 128          # 128-token chunks per batch (16)
NJ = T // 512          # q 512-windows per batch (4)
NFT = D_MODEL // 512   # output feature tiles (4)
FPC = HPC * DH         # per-core qkv feature width (256)


def build_nc(debug=False, reps=1):
    nc = bacc.Bacc("TRN2", target_bir_lowering=False, debug=False,
                   num_devices=NCORES)

    xT = nc.dram_tensor("xT", [D_IN, BT], BF16, kind="ExternalInput")
    wq = nc.dram_tensor("wq", [D_IN, FPC], BF16, kind="ExternalInput")
    wk = nc.dram_tensor("wk", [D_IN, FPC], BF16, kind="ExternalInput")
    wv = nc.dram_tensor("wv", [D_IN, FPC], BF16, kind="ExternalInput")
    bq = nc.dram_tensor("bq", [FPC], F32, kind="ExternalInput")
    bk = nc.dram_tensor("bk", [FPC], F32, kind="ExternalInput")
    bvb = nc.dram_tensor("bvb", [128, FPC], F32, kind="ExternalInput")
    wo = nc.dram_tensor("wo", [FPC, D_MODEL], BF16, kind="ExternalInput")
    cosT = nc.dram_tensor("cosT", [DH, T], BF16, kind="ExternalInput")
    sinTs = nc.dram_tensor("sinTs", [DH, T], BF16, kind="ExternalInput")
    y = nc.dram_tensor("y", [BT, D_MODEL], BF16, kind="ExternalOutput")

    dbg = {}
    if debug:
        dbg["qT"] = nc.dram_tensor("dbg_qT", [B, HPC, DH, T], BF16, kind="ExternalOutput")
        dbg["kT"] = nc.dram_tensor("dbg_kT", [B, HPC, DH, T], BF16, kind="ExternalOutput")
        dbg["v"] = nc.dram_tensor("dbg_v", [B, 128, NQ, FPC], BF16, kind="ExternalOutput")
        dbg["ot"] = nc.dram_tensor("dbg_ot", [B, NJ, HPC, DH, TOKT], BF16, kind="ExternalOutput")

    with tile.TileContext(nc) as tc:
        with (
            tc.tile_pool(name="persist", bufs=1) as pp,
            tc.tile_pool(name="weights", bufs=1) as wp,
            tc.tile_pool(name="qkv", bufs=1) as qp,
        ):
            # ---- per-core weights (persistent, loaded once)
            wq_sb = wp.tile([128, NDCH, FPC], BF16, name="wq_sb")
            wk_sb = wp.tile([128, NDCH, FPC], BF16, name="wk_sb")
            wv_sb = wp.tile([128, NDCH, FPC], BF16, name="wv_sb")
            for t_, d_ in ((wq_sb, wq), (wk_sb, wk), (wv_sb, wv)):
                nc.sync.dma_start(
                    t_[:], d_.ap().rearrange("(c p) f -> p c f", p=128))
            wo_sb = wp.tile([128, HPC, D_MODEL], BF16, name="wo_sb")
            nc.sync.dma_start(wo_sb[:],
                              wo.ap().rearrange("(h p) f -> p h f", p=128))

            # ---- constants
            cos_sb = pp.tile([DH, T], BF16, name="cos_sb")
            sin_sb = pp.tile([DH, T], BF16, name="sin_sb")
            nc.sync.dma_start(cos_sb[:], cosT.ap())
            nc.sync.dma_start(sin_sb[:], sinTs.ap())
            ones1_f = pp.tile([1, 128], F32, name="ones1_f")
            nc.gpsimd.memset(ones1_f[:], 1.0)
            ones1 = pp.tile([1, 128], F32R, name="ones1")
            nc.scalar.copy(ones1[:], ones1_f[:])
            onescol_f = pp.tile([128, 1], F32, name="onescol_f")
            nc.gpsimd.memset(onescol_f[:], 1.0)
            onescol = pp.tile([128, 1], BF16, name="onescol")
            nc.scalar.copy(onescol[:], onescol_f[:])
            bqt = pp.tile([128, HPC], F32, name="bqt")
            bkt = pp.tile([128, HPC], F32, name="bkt")
            nc.sync.dma_start(bqt[:], bq.ap().rearrange("(h d) -> d h", d=DH))
            nc.sync.dma_start(bkt[:], bk.ap().rearrange("(h d) -> d h", d=DH))
            bvb_sb = pp.tile([128, FPC], F32, name="bvb_sb")
            nc.sync.dma_start(bvb_sb[:], bvb.ap())

            # ---- per-batch Q^T/K^T/V buffers (persistent slots)
            qT_sb = [[qp.tile([DH, T], BF16, name=f"qT{b}_{h}") for h in range(HPC)]
                     for b in range(B)]
            kT_sb = [[qp.tile([DH, T], BF16, name=f"kT{b}_{h}") for h in range(HPC)]
                     for b in range(B)]
            v_sb = [qp.tile([128, NQ, FPC], BF16, name=f"v_sb{b}")
                    for b in range(B)]

            import contextlib
            rep_ctx = (tc.For_i(0, reps, 1, hint_engines=(
                mybir.EngineType.PE, mybir.EngineType.Activation,
                mybir.EngineType.DVE, mybir.EngineType.Pool,
                mybir.EngineType.SP))
                if reps > 1 else contextlib.nullcontext())
            with rep_ctx:
                _emit_body(nc, tc, xT, wq_sb, wk_sb, wv_sb, wo_sb, bqt, bkt,
                           bvb_sb, cos_sb, sin_sb, qT_sb, kT_sb, v_sb, y,
                           ones1, onescol, dbg)
    nc.compile()
    return nc


def _emit_body(nc, tc, xT, wq_sb, wk_sb, wv_sb, wo_sb, bqt, bkt, bvb_sb,
               cos_sb, sin_sb, qT_sb, kT_sb, v_sb, y, ones1, onescol, dbg):
    # ---- stage 1 for both batches (PE-dense, solo) -----------------------
    with (
        tc.tile_pool(name="xs", bufs=5) as xs,
        tc.tile_pool(name="st", bufs=4) as st,
        tc.tile_pool(name="ps_qk", bufs=4, space="PSUM") as psqk,
        tc.tile_pool(name="ps_v", bufs=4, space="PSUM") as psv,
    ):
        for b in range(B):
            _stage1(nc, b, xT, xs, st, psqk, psv, wq_sb, wk_sb, wv_sb,
                    bqt, bkt, bvb_sb, cos_sb, sin_sb, qT_sb, kT_sb, v_sb)
            if dbg:
                for h in range(HPC):
                    nc.sync.dma_start(dbg["qT"].ap()[b, h], qT_sb[b][h][:])
                    nc.sync.dma_start(dbg["kT"].ap()[b, h], kT_sb[b][h][:])
                nc.sync.dma_start(dbg["v"].ap()[b], v_sb[b][:])

    # ---- stage 2 + interleaved output projection -------------------------
    import itertools
    with (
        tc.tile_pool(name="sps", bufs=2, space="PSUM") as spsB,
        tc.tile_pool(name="ops", bufs=2, space="PSUM") as ops,
        tc.tile_pool(name="y_ps", bufs=3, space="PSUM") as yps,
        tc.tile_pool(name="pt_p", bufs=6) as ptp,
        tc.tile_pool(name="z_p", bufs=2) as zp,
        tc.tile_pool(name="scr", bufs=2) as scr,
        tc.tile_pool(name="rb_p", bufs=2) as rbp,
        tc.tile_pool(name="ot_p", bufs=16) as otp,
        tc.tile_pool(name="y_st", bufs=6) as ystg,
    ):
        # PSUM is only readable by ACT and DVE (not GPSIMD)
        evict_cycle = itertools.cycle(
            [nc.scalar.copy, nc.vector.tensor_copy])
        equeue = []

        def make_etile(b, j, ot_tiles, tl, ft):
            def emit():
                ps = yps.tile([128, TOKT], F32, name="y_ps")
                for h in range(HPC):
                    nc.tensor.matmul(
                        ps[:], ot_tiles[h][:, tl * 128:(tl + 1) * 128],
                        wo_sb[:, h, ft * TOKT:(ft + 1) * TOKT],
                        start=(h == 0), stop=(h == HPC - 1))
                yt = ystg.tile([128, TOKT], BF16, name="y_t")
                next(evict_cycle)(yt[:], ps[:])
                tt = j * 4 + tl
                nc.sync.dma_start(
                    y.ap()[b * T + tt * 128:b * T + (tt + 1) * 128,
                           ft * TOKT:(ft + 1) * TOKT], yt[:])
            return emit

        def fill(n):
            for _ in range(min(n, len(equeue))):
                equeue.pop(0)()

        def j_done(b, j, ot_tiles):
            if dbg:
                for h in range(HPC):
                    nc.sync.dma_start(dbg["ot"].ap()[b, j, h], ot_tiles[h][:])
            for tl in range(4):
                for ft in range(NFT):
                    equeue.append(make_etile(b, j, ot_tiles, tl, ft))

        for b in range(B):
            _stage2(nc, b, qT_sb, kT_sb, v_sb, onescol, ones1,
                    spsB, ops, ptp, zp, scr, rbp, otp, fill, j_done)
        while equeue:
            equeue.pop(0)()


def _stage1(nc, b, xT, xs, st, psqk, psv, wq_sb, wk_sb, wv_sb,
            bqt, bkt, bvb_sb, cos_sb, sin_sb, qT_sb, kT_sb, v_sb):
    """QKV projection + RoPE for batch b (all-bf16 matmuls).

    Q^T/K^T per head: W chunks stationary, x^T moving (N=512).
    V natural [tok, feat]: x^T chunks stationary, W_v moving (N=256),
    so P@V in stage 2 needs no transposes.
    """
    for tau in range(NTT):
        pos = tau * TOKT
        gtok = b * T + pos
        xts = []
        for quarter in range(4):
            xt = xs.tile([128, 4, TOKT], BF16, name="xt")
            nc.sync.dma_start(
                xt[:],
                xT.ap()[quarter * 512:(quarter + 1) * 512, gtok:gtok + TOKT]
                .rearrange("(c p) t -> p c t", p=128))
            xts.append(xt)

        def xch(c):
            return xts[c // 4][:, c % 4, :]

        for h in range(HPC):
            accq = psqk.tile([128, TOKT], F32, name="qk_acc")
            acck = psqk.tile([128, TOKT], F32, name="qk_acc")
            for c in range(NDCH):
                nc.tensor.matmul(accq[:], wq_sb[:, c, h * DH:(h + 1) * DH],
                                 xch(c), start=(c == 0), stop=(c == NDCH - 1))
                nc.tensor.matmul(acck[:], wk_sb[:, c, h * DH:(h + 1) * DH],
                                 xch(c), start=(c == 0), stop=(c == NDCH - 1))
            for acc, bias, dest in ((accq, bqt, qT_sb), (acck, bkt, kT_sb)):
                stg = st.tile([128, TOKT], BF16, name="stg")
                nc.scalar.activation(stg[:], acc[:], AF.Identity,
                                     bias=bias[:, h:h + 1], scale=1.0)
                rot = st.tile([128, TOKT], BF16, name="stg")
                nc.vector.tensor_copy(rot[0:64, :], stg[64:128, :])
                nc.vector.tensor_copy(rot[64:128, :], stg[0:64, :])
                nc.vector.tensor_tensor(
                    stg[:], stg[:], cos_sb[:, pos:pos + TOKT], ALU.mult)
                nc.vector.tensor_tensor(
                    rot[:], rot[:], sin_sb[:, pos:pos + TOKT], ALU.mult)
                nc.vector.tensor_tensor(
                    dest[b][h][:, pos:pos + TOKT], stg[:], rot[:], ALU.add)

        # V natural layout: two half-passes of 2 token chunks each; every
        # accumulation group gets its own PSUM tile (zero regions are
        # bank-wide, so groups must not share a bank).
        for half in range(2):
            accs = [psv.tile([128, FPC], F32, name="v_acc") for _ in range(2)]
            for c in range(NDCH):
                for t2 in range(2):
                    tck = half * 2 + t2
                    nc.tensor.matmul(
                        accs[t2][:],
                        xts[c // 4][:, c % 4, tck * 128:(tck + 1) * 128],
                        wv_sb[:, c, :],
                        start=(c == 0), stop=(c == NDCH - 1))
            for t2 in range(2):
                tck = half * 2 + t2
                nc.vector.tensor_tensor(
                    v_sb[b][:, tau * 4 + tck, :],
                    accs[t2][:], bvb_sb[:], ALU.add)


def _stage2(nc, b, qT_sb, kT_sb, v_sb, onescol, ones1,
            spsB, ops, ptp, zp, scr, rbp, otp, fill, j_done):
    """Causal attention for batch b, head-major: produces normalized O^T
    tiles per (j, head) and hands them to j_done for output projection.

    S^T tile -> exp (ACT, bf16) -> causal triangle zero (GPSIMD, diag
    chunks only) -> P@V accumulation (PE); row-sums via Z += P on DVE,
    reduced by one ones-matmul per (j, head). Diagonal tiles trimmed to
    the valid q-range. fill() interleaves pending output-projection tiles.
    """
    for j in range(NJ):
        nkk = 4 * j + 4
        ot_tiles = []
        for h in range(HPC):
            op = ops.tile([128, TOKT], F32, name="o_ps")
            Z = zp.tile([128, TOKT], BF16, name="zt")
            for kk in range(nkk):
                diag = (kk // 4 == j)
                q0 = (kk % 4) * 128 if diag else 0
                qs = slice(q0, TOKT)
                sp = spsB.tile([128, TOKT], F32, name="st_ps")
                nc.tensor.matmul(sp[:, qs], kT_sb[b][h][:, kk * 128:(kk + 1) * 128],
                                 qT_sb[b][h][:, j * TOKT + q0:(j + 1) * TOKT],
                                 start=True, stop=True)
                pt = ptp.tile([128, TOKT], BF16, name="pt")
                nc.scalar.activation(pt[:, qs], sp[:, qs], AF.Exp, bias=0.0,
                                     scale=SCALE)
                if diag:
                    # zero entries with q < k on the triangular chunk:
                    # keep where f - p >= 0
                    nc.gpsimd.affine_select(
                        out=pt[:, q0:q0 + 128], in_=pt[:, q0:q0 + 128],
                        compare_op=ALU.is_ge, fill=0.0, base=0,
                        pattern=[[1, 128]], channel_multiplier=-1)
                nc.tensor.matmul(op[:, qs], v_sb[b][:, kk, h * DH:(h + 1) * DH],
                                 pt[:, qs], start=(kk == 0), stop=(kk == nkk - 1))
                if kk == 0:
                    nc.vector.tensor_copy(Z[:], pt[:])
                else:
                    nc.vector.tensor_tensor(Z[:, qs], Z[:, qs], pt[:, qs],
                                            ALU.add)
                fill(2)
            # rowsum = ones^T Z -> reciprocal -> broadcast -> normalize
            rps = spsB.tile([1, TOKT], F32, name="st_ps")
            nc.tensor.matmul(rps[:], onescol[:], Z[:], start=True, stop=True)
            fill(1)
            rinv = scr.tile([1, TOKT], F32R, name="rinv")
            with nc.allow_low_precision(reason="f32r storage is f32-width"):
                nc.vector.reciprocal(rinv[:], rps[:])
            rb_ps = spsB.tile([128, TOKT], F32, name="st_ps")
            nc.tensor.matmul(rb_ps[:], ones1[:], rinv[:], start=True, stop=True)
            fill(1)
            rb = rbp.tile([128, TOKT], F32, name="rb")
            nc.scalar.copy(rb[:], rb_ps[:])
            ot = otp.tile([DH, TOKT], BF16, name="ot")
            nc.vector.tensor_tensor(ot[:], op[:], rb[:], ALU.mult)
            ot_tiles.append(ot)
        j_done(b, j, ot_tiles)


_CACHE = {}


def _get_nc():
    if "nc" not in _CACHE:
        _CACHE["nc"] = build_nc(debug=bool(int(os.environ.get("KERNEL_DEBUG", "0"))))
    return _CACHE["nc"]


def _host_prep(x, W_qkv, b_qkv, W_out, mask):
    bf16 = mybir.dt.np(BF16)
    xT = np.ascontiguousarray(x.reshape(BT, D_IN).T.astype(bf16))
    Wr = W_qkv.reshape(D_IN, H, 3, DH)
    br = b_qkv.reshape(H, 3, DH)
    # RoPE tables, transposed, sign-folded (rows 0:64 of sinTs negated)
    inv_freq = (1.0 / (10000.0 ** (np.arange(0, DH, 2, dtype=np.float32) / DH))).astype(np.float32)
    tpos = np.arange(T, dtype=np.float32)
    freqs = tpos[:, None] * inv_freq[None, :]              # (T, 64)
    emb = np.concatenate([freqs, freqs], axis=-1)          # (T, 128)
    cosT = np.ascontiguousarray(np.cos(emb).T.astype(bf16))
    sinT = np.sin(emb).astype(np.float32).T
    sinTs = sinT.copy()
    sinTs[0:64] = -sinTs[0:64]
    sinTs = np.ascontiguousarray(sinTs.astype(bf16))

    in_maps = []
    for i in range(NCORES):
        hs = [HPC * i + k for k in range(HPC)]
        bv = np.ascontiguousarray(br[hs, 2, :].reshape(FPC).astype(np.float32))
        in_maps.append({
            "xT": xT,
            "wq": np.ascontiguousarray(Wr[:, hs, 0, :].reshape(D_IN, FPC).astype(bf16)),
            "wk": np.ascontiguousarray(Wr[:, hs, 1, :].reshape(D_IN, FPC).astype(bf16)),
            "wv": np.ascontiguousarray(Wr[:, hs, 2, :].reshape(D_IN, FPC).astype(bf16)),
            "bq": np.ascontiguousarray(br[hs, 0, :].reshape(FPC).astype(np.float32)),
            "bk": np.ascontiguousarray(br[hs, 1, :].reshape(FPC).astype(np.float32)),
            "bvb": np.ascontiguousarray(np.broadcast_to(bv, (128, FPC)).copy()),
            "wo": np.ascontiguousarray(W_out[hs[0] * DH:(hs[-1] + 1) * DH, :].astype(bf16)),
            "cosT": cosT,
            "sinTs": sinTs,
        })
    return in_maps


def kernel(x, W_qkv, b_qkv, W_out, b_out, mask):
    x = np.asarray(x, dtype=np.float32)
    in_maps = _host_prep(np.asarray(x), np.asarray(W_qkv), np.asarray(b_qkv),
                         np.asarray(W_out), np.asarray(mask))
    nc = _get_nc()
    res = run_bass_kernel_spmd(nc, in_maps, core_ids=list(range(NCORES)))
    out = res.results[0]["y"].astype(np.float32)
    for i in range(1, NCORES):
        out += res.results[i]["y"].astype(np.float32)
    out += np.asarray(b_out, dtype=np.float32)[None, :]
    return out.reshape(B, T, D_MODEL).astype(np.float32)


# revision 18
# speedup vs baseline: 1.4189x; 1.0560x over previous
"""Multi-head causal attention (B=2, T=2048, D=2048, H=16) on 8 trn2 NeuronCores.

Sharding: tensor-parallel over heads (2 heads/core). x^T replicated, W_qkv
column-sliced and W_out row-sliced per core; each core computes a full-shape
bf16 partial of the output projection and the host sums the 8 partials
(+ b_out) in f32.

v2 design (vs f32r baseline):
- All matmul operands bf16 (full PE rate at any free size; f32 PSUM accum).
  Halves DMA + SBUF traffic; rel-err budget (2e-2) allows it.
- V computed in natural [tok, feat] layout directly (x-chunk stationary,
  W_v moving) -- no PE transposes.
- Softmax row-sums accumulated on DVE (bf16 Z += P tile, 2x perf mode)
  instead of per-tile ones-matmuls on PE; one ones-matmul per (j, head)
  reduces Z across partitions.
- Diagonal score tiles trimmed: only the valid q-range is matmul'd/exp'd,
  and the causal affine_select shrinks to one [128,128] triangular chunk.
- Output projection tiles are emitted interleaved into stage2 as each
  j-block's O^T finishes, keeping PE busy while ACT runs the exps; y is
  evicted to bf16 (rotating ACT/DVE/Pool) and DMA'd per tile.
"""

import math
import os

import numpy as np

import concourse.bass as bass
import concourse.mybir as mybir
import concourse.tile as tile
from concourse import bacc
from concourse.bass_utils import run_bass_kernel_spmd

B, T, D_IN, D_MODEL, H = 2, 2048, 2048, 2048, 16
DH = 128
NCORES = 8
HPC = H // NCORES  # heads per core
BT = B * T
SCALE = 1.0 / math.sqrt(DH)

F32 = mybir.dt.float32
F32R = mybir.dt.float32r
BF16 = mybir.dt.bfloat16
AF = mybir.ActivationFunctionType
ALU = mybir.AluOpType

TOKT = 512             # token tile (q-window, stage-1 tile)
NTT = T // TOKT        # token tiles per batch (4)
NDCH = D_IN // 128     # d_in contraction chunks (16)
NQ = T // 128          # 128-token chunks per batch (16)
NJ = T // 512          # q 512-windows per batch (4)
NFT = D_MODEL // 512   # output feature tiles (4)
FPC = HPC * DH         # per-core qkv feature width (256)


def build_nc(debug=False, reps=1):
    nc = bacc.Bacc("TRN2", target_bir_lowering=False, debug=False,
                   num_devices=NCORES)

    xT = nc.dram_tensor("xT", [D_IN, BT], BF16, kind="ExternalInput")
    wq = nc.dram_tensor("wq", [D_IN, FPC], BF16, kind="ExternalInput")
    wk = nc.dram_tensor("wk", [D_IN, FPC], BF16, kind="ExternalInput")
    wv = nc.dram_tensor("wv", [D_IN, FPC], BF16, kind="ExternalInput")
    bq = nc.dram_tensor("bq", [FPC], F32, kind="ExternalInput")
    bk = nc.dram_tensor("bk", [FPC], F32, kind="ExternalInput")
    bvb = nc.dram_tensor("bvb", [128, FPC], F32, kind="ExternalInput")
    wo = nc.dram_tensor("wo", [FPC, D_MODEL], BF16, kind="ExternalInput")
    cosT = nc.dram_tensor("cosT", [DH, T], BF16, kind="ExternalInput")
    sinTs = nc.dram_tensor("sinTs", [DH, T], BF16, kind="ExternalInput")
    y = nc.dram_tensor("y", [BT, D_MODEL], BF16, kind="ExternalOutput")

    dbg = {}
    if debug:
        dbg["qT"] = nc.dram_tensor("dbg_qT", [B, HPC, DH, T], BF16, kind="ExternalOutput")
        dbg["kT"] = nc.dram_tensor("dbg_kT", [B, HPC, DH, T], BF16, kind="ExternalOutput")
        dbg["v"] = nc.dram_tensor("dbg_v", [B, 128, NQ, FPC], BF16, kind="ExternalOutput")
        dbg["ot"] = nc.dram_tensor("dbg_ot", [B, NJ, HPC, DH, TOKT], BF16, kind="ExternalOutput")

    with tile.TileContext(nc) as tc:
        with (
            tc.tile_pool(name="persist", bufs=1) as pp,
            tc.tile_pool(name="weights", bufs=1) as wp,
            tc.tile_pool(name="qkv", bufs=1) as qp,
        ):
            # ---- per-core weights (persistent, loaded once)
            wq_sb = wp.tile([128, NDCH, FPC], BF16, name="wq_sb")
            wk_sb = wp.tile([128, NDCH, FPC], BF16, name="wk_sb")
            wv_sb = wp.tile([128, NDCH, FPC], BF16, name="wv_sb")
            for t_, d_ in ((wq_sb, wq), (wk_sb, wk), (wv_sb, wv)):
                nc.sync.dma_start(
                    t_[:], d_.ap().rearrange("(c p) f -> p c f", p=128))
            wo_sb = wp.tile([128, HPC, D_MODEL], BF16, name="wo_sb")
            nc.sync.dma_start(wo_sb[:],
                              wo.ap().rearrange("(h p) f -> p h f", p=128))

            # ---- constants
            cos_sb = pp.tile([DH, T], BF16, name="cos_sb")
            sin_sb = pp.tile([DH, T], BF16, name="sin_sb")
            nc.sync.dma_start(cos_sb[:], cosT.ap())
            nc.sync.dma_start(sin_sb[:], sinTs.ap())
            ones1_f = pp.tile([1, 128], F32, name="ones1_f")
            nc.gpsimd.memset(ones1_f[:], 1.0)
            ones1 = pp.tile([1, 128], F32R, name="ones1")
            nc.scalar.copy(ones1[:], ones1_f[:])
            onescol_f = pp.tile([128, 1], F32, name="onescol_f")
            nc.gpsimd.memset(onescol_f[:], 1.0)
            onescol = pp.tile([128, 1], BF16, name="onescol")
            nc.scalar.copy(onescol[:], onescol_f[:])
            bqt = pp.tile([128, HPC], F32, name="bqt")
            bkt = pp.tile([128, HPC], F32, name="bkt")
            nc.sync.dma_start(bqt[:], bq.ap().rearrange("(h d) -> d h", d=DH))
            nc.sync.dma_start(bkt[:], bk.ap().rearrange("(h d) -> d h", d=DH))
            bvb_sb = pp.tile([128, FPC], F32, name="bvb_sb")
            nc.sync.dma_start(bvb_sb[:], bvb.ap())

            # ---- per-batch Q^T/K^T/V buffers (persistent slots)
            qT_sb = [[qp.tile([DH, T], BF16, name=f"qT{b}_{h}") for h in range(HPC)]
                     for b in range(B)]
            kT_sb = [[qp.tile([DH, T], BF16, name=f"kT{b}_{h}") for h in range(HPC)]
                     for b in range(B)]
            v_sb = [qp.tile([128, NQ, FPC], BF16, name=f"v_sb{b}")
                    for b in range(B)]

            import contextlib
            rep_ctx = (tc.For_i(0, reps, 1, hint_engines=(
                mybir.EngineType.PE, mybir.EngineType.Activation,
                mybir.EngineType.DVE, mybir.EngineType.Pool,
                mybir.EngineType.SP))
                if reps > 1 else contextlib.nullcontext())
            with rep_ctx:
                _emit_body(nc, tc, xT, wq_sb, wk_sb, wv_sb, wo_sb, bqt, bkt,
                           bvb_sb, cos_sb, sin_sb, qT_sb, kT_sb, v_sb, y,
                           ones1, onescol, dbg)
    nc.compile()
    return nc


def _emit_body(nc, tc, xT, wq_sb, wk_sb, wv_sb, wo_sb, bqt, bkt, bvb_sb,
               cos_sb, sin_sb, qT_sb, kT_sb, v_sb, y, ones1, onescol, dbg):
    import itertools

    equeue = []
    # PSUM is only readable by ACT and DVE (not GPSIMD)
    evict_cycle = itertools.cycle(
        [nc.vector.tensor_copy, nc.vector.tensor_copy,
         nc.vector.tensor_copy, nc.scalar.copy])

    def s1_done(b):
        if dbg:
            for h in range(HPC):
                nc.sync.dma_start(dbg["qT"].ap()[b, h], qT_sb[b][h][:])
                nc.sync.dma_start(dbg["kT"].ap()[b, h], kT_sb[b][h][:])
            nc.sync.dma_start(dbg["v"].ap()[b], v_sb[b][:])

    _YPS = [None]  # the live y-PSUM pool; set when phase B's pools open

    def j_done_factory(ystg):
        def make_eunit(b, j, ot_tiles, tl, ftp):
            def emit():
                # two ft tiles share one 2-bank PSUM tile (one group per
                # bank), one merged eviction + DMA
                ps = _YPS[0].tile([128, 2 * TOKT], F32, name="y_ps")
                for g in range(2):
                    ft = 2 * ftp + g
                    for h in range(HPC):
                        nc.tensor.matmul(
                            ps[:, g * TOKT:(g + 1) * TOKT],
                            ot_tiles[h][:, tl * 128:(tl + 1) * 128],
                            wo_sb[:, h, ft * TOKT:(ft + 1) * TOKT],
                            start=(h == 0), stop=(h == HPC - 1))
                yt = ystg.tile([128, 2 * TOKT], BF16, name="y_t")
                next(evict_cycle)(yt[:], ps[:])
                tt = j * 4 + tl
                nc.sync.dma_start(
                    y.ap()[b * T + tt * 128:b * T + (tt + 1) * 128,
                           ftp * 2 * TOKT:(ftp + 1) * 2 * TOKT], yt[:])
            return emit

        def j_done(b, j, ot_tiles):
            if dbg:
                for h in range(HPC):
                    nc.sync.dma_start(dbg["ot"].ap()[b, j, h], ot_tiles[h][:])
            for tl in range(4):
                for ftp in range(NFT // 2):
                    equeue.append(make_eunit(b, j, ot_tiles, tl, ftp))
        return j_done

    # stage-2 SBUF pools live for the whole body
    with (
        tc.tile_pool(name="pt_p", bufs=6) as ptp,
        tc.tile_pool(name="z_p", bufs=2) as zp,
        tc.tile_pool(name="scr", bufs=2) as scr,
        tc.tile_pool(name="rb_p", bufs=2) as rbp,
        tc.tile_pool(name="ot_p", bufs=16) as otp,
        tc.tile_pool(name="y_st", bufs=4) as ystg,
    ):
        # ---- phase A: stage1(b0); then stage2(b0) with stage1(b1) chunks
        # interleaved as PE filler (E units can't run yet: no free PSUM).
        with (
            tc.tile_pool(name="xs", bufs=5) as xs,
            tc.tile_pool(name="st", bufs=4) as st,
            tc.tile_pool(name="ps_qk", bufs=2, space="PSUM") as psqk,
            tc.tile_pool(name="ps_v", bufs=2, space="PSUM") as psv,
            tc.tile_pool(name="sps", bufs=2, space="PSUM") as spsB,
            tc.tile_pool(name="ops", bufs=2, space="PSUM") as ops,
        ):
            for _ in _stage1_gen(nc, 0, xT, xs, st, psqk, psv, wq_sb, wk_sb,
                                 wv_sb, bqt, bkt, bvb_sb, cos_sb, sin_sb,
                                 qT_sb, kT_sb, v_sb):
                pass
            s1_done(0)

            s1b1 = _stage1_gen(nc, 1, xT, xs, st, psqk, psv, wq_sb, wk_sb,
                               wv_sb, bqt, bkt, bvb_sb, cos_sb, sin_sb,
                               qT_sb, kT_sb, v_sb)

            def fill_a(n, reserve=0):
                for _ in range(n):
                    if next(s1b1, StopIteration) is StopIteration:
                        break

            _stage2(nc, 0, qT_sb, kT_sb, v_sb, onescol, ones1,
                    spsB, ops, ptp, zp, scr, rbp, otp, fill_a,
                    j_done_factory(ystg))
            for _ in s1b1:
                pass
            s1_done(1)

        # ---- phase B: stage2(b1) with output-projection units interleaved
        with (
            tc.tile_pool(name="sps", bufs=2, space="PSUM") as spsB,
            tc.tile_pool(name="ops", bufs=2, space="PSUM") as ops,
            tc.tile_pool(name="y_ps", bufs=2, space="PSUM") as yps,
        ):
            jd = j_done_factory(ystg)
            _YPS[0] = yps

            def fill_b(n, reserve=0):
                for _ in range(n):
                    if len(equeue) <= reserve:
                        break
                    equeue.pop(0)()

            _stage2(nc, 1, qT_sb, kT_sb, v_sb, onescol, ones1,
                    spsB, ops, ptp, zp, scr, rbp, otp, fill_b, jd)
            while equeue:
                equeue.pop(0)()


def _stage1_gen(nc, b, xT, xs, st, psqk, psv, wq_sb, wk_sb, wv_sb,
                bqt, bkt, bvb_sb, cos_sb, sin_sb, qT_sb, kT_sb, v_sb):
    """QKV projection + RoPE for batch b (all-bf16 matmuls), as a
    generator yielding after each ~2-matmul chunk so it can be
    interleaved as PE filler into stage 2 of the other batch.

    Q^T/K^T per head: W chunks stationary, x^T moving (N=512).
    V natural [tok, feat]: x^T chunks stationary, W_v moving (N=256),
    so P@V in stage 2 needs no transposes.
    """
    for tau in range(NTT):
        pos = tau * TOKT
        gtok = b * T + pos
        xts = []
        for quarter in range(4):
            xt = xs.tile([128, 4, TOKT], BF16, name="xt")
            nc.sync.dma_start(
                xt[:],
                xT.ap()[quarter * 512:(quarter + 1) * 512, gtok:gtok + TOKT]
                .rearrange("(c p) t -> p c t", p=128))
            xts.append(xt)

        def xch(c):
            return xts[c // 4][:, c % 4, :]

        def qk_pass(h):
            accq = psqk.tile([128, TOKT], F32, name="qk_acc")
            acck = psqk.tile([128, TOKT], F32, name="qk_acc")
            for c in range(NDCH):
                nc.tensor.matmul(accq[:], wq_sb[:, c, h * DH:(h + 1) * DH],
                                 xch(c), start=(c == 0), stop=(c == NDCH - 1))
                nc.tensor.matmul(acck[:], wk_sb[:, c, h * DH:(h + 1) * DH],
                                 xch(c), start=(c == 0), stop=(c == NDCH - 1))
                yield
            for acc, bias, dest in ((accq, bqt, qT_sb), (acck, bkt, kT_sb)):
                stg = st.tile([128, TOKT], BF16, name="stg")
                nc.scalar.activation(stg[:], acc[:], AF.Identity,
                                     bias=bias[:, h:h + 1], scale=1.0)
                rot = st.tile([128, TOKT], BF16, name="stg")
                nc.vector.tensor_copy(rot[0:64, :], stg[64:128, :])
                nc.vector.tensor_copy(rot[64:128, :], stg[0:64, :])
                nc.vector.tensor_tensor(
                    stg[:], stg[:], cos_sb[:, pos:pos + TOKT], ALU.mult)
                nc.vector.tensor_tensor(
                    rot[:], rot[:], sin_sb[:, pos:pos + TOKT], ALU.mult)
                nc.vector.tensor_tensor(
                    dest[b][h][:, pos:pos + TOKT], stg[:], rot[:], ALU.add)

        def v_half(half):
            # V natural layout: 2 token chunks per half-pass; every
            # accumulation group gets its own full-bank PSUM tile (zero
            # regions are bank-wide, so groups must not share a bank).
            accs = [psv.tile([128, TOKT], F32, name="v_acc") for _ in range(2)]
            for c in range(NDCH):
                for t2 in range(2):
                    tck = half * 2 + t2
                    nc.tensor.matmul(
                        accs[t2][:, 0:FPC],
                        xts[c // 4][:, c % 4, tck * 128:(tck + 1) * 128],
                        wv_sb[:, c, :],
                        start=(c == 0), stop=(c == NDCH - 1))
                yield
            for t2 in range(2):
                tck = half * 2 + t2
                nc.vector.tensor_tensor(
                    v_sb[b][:, tau * 4 + tck, :],
                    accs[t2][:, 0:FPC], bvb_sb[:], ALU.add)

        # interleave so a head-pass's PSUM eviction hides under the next
        # V half-pass (and vice versa) with only 2+2 PSUM banks
        yield from qk_pass(0)
        yield from v_half(0)
        yield from qk_pass(1)
        yield from v_half(1)


def _stage2(nc, b, qT_sb, kT_sb, v_sb, onescol, ones1,
            spsB, ops, ptp, zp, scr, rbp, otp, fill, j_done):
    """Causal attention for batch b, head-major: produces normalized O^T
    tiles per (j, head) and hands them to j_done for output projection.

    S^T tile -> exp (ACT, bf16) -> causal triangle zero (GPSIMD, diag
    chunks only) -> P@V accumulation (PE); row-sums via Z += P on DVE,
    reduced by one ones-matmul per (j, head). Diagonal tiles trimmed to
    the valid q-range. fill() interleaves pending output-projection tiles.
    """
    for j in range(NJ):
        nkk = 4 * j + 4
        ot_tiles = []
        for h in range(HPC):
            op = ops.tile([128, TOKT], F32, name="o_ps")
            Z = zp.tile([128, TOKT], BF16, name="zt")
            for kk in range(nkk):
                diag = (kk // 4 == j)
                q0 = (kk % 4) * 128 if diag else 0
                qs = slice(q0, TOKT)
                sp = spsB.tile([128, TOKT], F32, name="st_ps")
                nc.tensor.matmul(sp[:, qs], kT_sb[b][h][:, kk * 128:(kk + 1) * 128],
                                 qT_sb[b][h][:, j * TOKT + q0:(j + 1) * TOKT],
                                 start=True, stop=True)
                # kk==0 writes P straight into Z (it doubles as the running
                # row-sum accumulator), saving a copy
                pt = Z if kk == 0 else ptp.tile([128, TOKT], BF16, name="pt")
                nc.scalar.activation(pt[:, qs], sp[:, qs], AF.Exp, bias=0.0,
                                     scale=SCALE)
                if diag:
                    # zero entries with q < k on the triangular chunk:
                    # keep where f - p >= 0
                    nc.gpsimd.affine_select(
                        out=pt[:, q0:q0 + 128], in_=pt[:, q0:q0 + 128],
                        compare_op=ALU.is_ge, fill=0.0, base=0,
                        pattern=[[1, 128]], channel_multiplier=-1)
                nc.tensor.matmul(op[:, qs], v_sb[b][:, kk, h * DH:(h + 1) * DH],
                                 pt[:, qs], start=(kk == 0), stop=(kk == nkk - 1))
                if kk != 0:
                    nc.vector.tensor_tensor(Z[:, qs], Z[:, qs], pt[:, qs],
                                            ALU.add)
                fill(1, reserve=4)
            # rowsum = ones^T Z -> reciprocal -> broadcast -> normalize
            rps = spsB.tile([1, TOKT], F32, name="st_ps")
            nc.tensor.matmul(rps[:], onescol[:], Z[:], start=True, stop=True)
            fill(2)
            rinv = scr.tile([1, TOKT], F32R, name="rinv")
            with nc.allow_low_precision(reason="f32r storage is f32-width"):
                nc.vector.reciprocal(rinv[:], rps[:])
            rb_ps = spsB.tile([128, TOKT], F32, name="st_ps")
            nc.tensor.matmul(rb_ps[:], ones1[:], rinv[:], start=True, stop=True)
            fill(1)
            rb = rbp.tile([128, TOKT], F32, name="rb")
            nc.scalar.copy(rb[:], rb_ps[:])
            ot = otp.tile([DH, TOKT], BF16, name="ot")
            nc.vector.tensor_tensor(ot[:], op[:], rb[:], ALU.mult)
            ot_tiles.append(ot)
        j_done(b, j, ot_tiles)


_CACHE = {}


def _get_nc():
    if "nc" not in _CACHE:
        _CACHE["nc"] = build_nc(debug=bool(int(os.environ.get("KERNEL_DEBUG", "0"))))
    return _CACHE["nc"]


def _host_prep(x, W_qkv, b_qkv, W_out, mask):
    bf16 = mybir.dt.np(BF16)
    xT = np.ascontiguousarray(x.reshape(BT, D_IN).T.astype(bf16))
    Wr = W_qkv.reshape(D_IN, H, 3, DH)
    br = b_qkv.reshape(H, 3, DH)
    # RoPE tables, transposed, sign-folded (rows 0:64 of sinTs negated)
    inv_freq = (1.0 / (10000.0 ** (np.arange(0, DH, 2, dtype=np.float32) / DH))).astype(np.float32)
    tpos = np.arange(T, dtype=np.float32)
    freqs = tpos[:, None] * inv_freq[None, :]              # (T, 64)
    emb = np.concatenate([freqs, freqs], axis=-1)          # (T, 128)
    cosT = np.ascontiguousarray(np.cos(emb).T.astype(bf16))
    sinT = np.sin(emb).astype(np.float32).T
    sinTs = sinT.copy()
    sinTs[0:64] = -sinTs[0:64]
    sinTs = np.ascontiguousarray(sinTs.astype(bf16))

    in_maps = []
    for i in range(NCORES):
        hs = [HPC * i + k for k in range(HPC)]
        bv = np.ascontiguousarray(br[hs, 2, :].reshape(FPC).astype(np.float32))
        in_maps.append({
            "xT": xT,
            "wq": np.ascontiguousarray(Wr[:, hs, 0, :].reshape(D_IN, FPC).astype(bf16)),
            "wk": np.ascontiguousarray(Wr[:, hs, 1, :].reshape(D_IN, FPC).astype(bf16)),
            "wv": np.ascontiguousarray(Wr[:, hs, 2, :].reshape(D_IN, FPC).astype(bf16)),
            "bq": np.ascontiguousarray(br[hs, 0, :].reshape(FPC).astype(np.float32)),
            "bk": np.ascontiguousarray(br[hs, 1, :].reshape(FPC).astype(np.float32)),
            "bvb": np.ascontiguousarray(np.broadcast_to(bv, (128, FPC)).copy()),
            "wo": np.ascontiguousarray(W_out[hs[0] * DH:(hs[-1] + 1) * DH, :].astype(bf16)),
            "cosT": cosT,
            "sinTs": sinTs,
        })
    return in_maps


def kernel(x, W_qkv, b_qkv, W_out, b_out, mask):
    x = np.asarray(x, dtype=np.float32)
    in_maps = _host_prep(np.asarray(x), np.asarray(W_qkv), np.asarray(b_qkv),
                         np.asarray(W_out), np.asarray(mask))
    nc = _get_nc()
    res = run_bass_kernel_spmd(nc, in_maps, core_ids=list(range(NCORES)))
    out = res.results[0]["y"].astype(np.float32)
    for i in range(1, NCORES):
        out += res.results[i]["y"].astype(np.float32)
    out += np.asarray(b_out, dtype=np.float32)[None, :]
    return out.reshape(B, T, D_MODEL).astype(np.float32)
